# revision 39
# baseline (speedup 1.0000x reference)
"""GCN link-prediction (3-layer GCNConv encode + dot-product decode) on 8 trn2
NeuronCores via Bass/Tile.

Strategy (self-contained; shapes hardcoded for the nn_Net_14963666059852 spec):
  - Reformulate GCNConv:  out = dis * (sum_{s->d, incl self} Hhat[s]) + b,
    where Hhat = (X @ W) * dis[:, None].  Aggregation is a binary-adjacency
    SpMM -> gather rows + segment-sum.
  - Shard nodes across 8 cores (12500 each).  Edges partitioned by dst core,
    sorted by (supertile of dst, src-quarter, dst tile); each (tile, quarter)
    segment padded to a multiple of 128 slots.  Chunk counts are commonized
    across cores so a single SPMD program serves all 8 cores.
  - Per 128-edge chunk: one-hot(edge -> dst slot) built on DVE from
    iota==dstloc; PE matmul (onehot.T @ messages) accumulates the segment-sum
    in PSUM.  Messages come from dma_gather (int16 idx, source table split in
    4 quarters of 25000 rows).
  - Per layer: each core computes Hhat for its own nodes, AllGather makes the
    full table visible to every core for next layer's gathers.
  - Decode: eval pairs sharded 50k/core, grouped by (quarter(a), quarter(b)),
    gather z rows for both sides, DVE mul + reduce_sum -> logits.
"""

import os as _os

import numpy as np

import concourse.bass as bass
import concourse.bacc as bacc
import concourse.tile as tile
import concourse.mybir as mybir
from concourse import library_config
from concourse.masks import make_identity
from concourse.bass_utils import run_bass_kernel_spmd

# ---------------- problem constants (hardcoded per spec) ----------------
N = 100000
NCORES = 8
NPC = N // NCORES          # 12500 nodes per core
P = 128
TPC = (NPC + P - 1) // P   # 98 tiles per core (last has 84 rows)
LAST_ROWS = NPC - (TPC - 1) * P  # 84
G = 4                      # tiles per supertile
NST = (TPC + G - 1) // G   # 25 supertiles
NQ = 4
QR = N // NQ               # 25000 rows per quarter table
IN_C, HID_C, OUT_C = 128, 256, 128
NEVAL_PC = 50000           # eval pairs per core
CAP = 8                    # chunks per gather call (1024-desc ring: one full call)
CAP_EV = 8                 # max chunks per gather call (decode)

F32 = mybir.dt.float32
I16 = mybir.dt.int16
I8 = mybir.dt.int8
BF16 = mybir.dt.bfloat16

# gather-table dtype config (numerics vs bandwidth)
GATHER_BF16 = True
Z_BF16 = True
GDT = mybir.dt.bfloat16 if GATHER_BF16 else F32
ZDT = mybir.dt.bfloat16 if Z_BF16 else F32


def _blob_layout(meta):
    """Single packed int16 input blob [P, C]: per-component column offsets.

    Components (i16 cols):
      xT    : [P, TPC*P] bf16 node features, transposed
      disT  : [P, TPC] f32 -> 2*TPC cols
      dstloc: [P, totch] i8 -> totch/2 cols
      idx   : [16, totch*8] i16 folded 8x -> [P, totch]
      ea/eb : [16, eacols/ebcols] folded 8x -> [P, eacols/8], [P, ebcols/8]
      W1    : [P, HID_C] bf16; W2: 2x[P, HID_C]; W3: 2x[P, OUT_C]
      bias  : row 0 only, b1|b2|b3 bf16 (stride-0 broadcast on device)
    """
    totch = meta["totch"]
    lay = {}
    o = 0
    def add(name, cols):
        nonlocal o
        o = (o + 63) // 64 * 64
        lay[name] = o
        o += cols
    add("xT", TPC * P)
    add("disT", 2 * TPC)
    assert totch % 2 == 0
    add("dstloc", totch // 2)
    assert meta["idxcols"] == totch * 8
    add("idx", totch)
    assert meta["eacols"] % 8 == 0 and meta["ebcols"] % 8 == 0
    add("ea", meta["eacols"] // 8)
    add("eb", meta["ebcols"] // 8)
    add("W1", HID_C)
    add("W2", 2 * HID_C)
    add("W3", 2 * OUT_C)
    add("bias", 2 * HID_C + OUT_C)
    lay["_total"] = (o + 63) // 64 * 64
    return lay


# ======================================================================
# host-side preprocessing
# ======================================================================

def _ceil_div(a, b):
    return (a + b - 1) // b


def _pack_idx16(slot_vals, calls, ncols):
    """Pack per-slot int16 indices into the [16, ncols] 16-wrapped layout.

    calls: list of (slot0, nslots, col0). Within a call, slot i ->
    [i % 16, col0 + i // 16]. (Replication into the 8 partition groups
    of the [128, ncols] SBUF tile happens on device.)
    """
    arr = np.zeros((16, ncols), np.int16)
    for slot0, nslots, col0 in calls:
        s = slot_vals[slot0 : slot0 + nslots]
        arr[:, col0 : col0 + nslots // 16] = s.reshape(-1, 16).T
    return arr


def _fold8(base16):
    """[16, 8w] -> [128, w] by stacking the 8 column chunks."""
    w = base16.shape[1] // 8
    return np.concatenate(
        [base16[:, j * w : (j + 1) * w] for j in range(8)], axis=0)


def _balance_perm(deg_counts):
    """old->new node permutation: serpentine-deal nodes (sorted by in-degree
    desc) across the 784 global tiles so every tile has near-equal edge load.
    Global tile g = c * TPC + t gets nodes new_id in
    [c * NPC + t * P, c * NPC + t * P + size), size = 128 (84 for t = 97)."""
    nbins = NCORES * TPC
    sizes = np.full(nbins, P, np.int64)
    sizes[TPC - 1 :: TPC] = LAST_ROWS
    base = np.zeros(nbins, np.int64)
    c_of = np.arange(nbins) // TPC
    t_of = np.arange(nbins) % TPC
    base = c_of * NPC + t_of * P

    order = np.argsort(-deg_counts, kind="stable")  # old ids, heavy first
    old2new = np.empty(N, np.int64)
    fill = np.zeros(nbins, np.int64)
    pos = 0
    rnd = 0
    while pos < N:
        bins = np.arange(nbins) if rnd % 2 == 0 else np.arange(nbins)[::-1]
        avail = bins[fill[bins] < sizes[bins]]
        take = min(len(avail), N - pos)
        avail = avail[:take]
        old2new[order[pos : pos + take]] = base[avail] + fill[avail]
        fill[avail] += 1
        pos += take
        rnd += 1
    return old2new


def _preprocess(x, edge_index, pos_edge_index, neg_edge_index):
    src0 = np.asarray(edge_index[0], dtype=np.int64)
    dst0 = np.asarray(edge_index[1], dtype=np.int64)

    deg_counts = np.bincount(dst0, minlength=N)
    deg = deg_counts.astype(np.float32) + np.float32(1.0)
    dis0 = (np.float32(1.0) / np.sqrt(deg)).astype(np.float32)

    # node permutation balancing per-tile edge counts across cores
    old2new = _balance_perm(deg_counts)
    new2old = np.empty(N, np.int64)
    new2old[old2new] = np.arange(N)

    src = old2new[src0]
    dst = old2new[dst0]
    x = np.asarray(x, dtype=np.float32)[new2old]
    dis = dis0[new2old]

    # augment with self edges
    arange_n = np.arange(N, dtype=np.int64)
    src_a = np.concatenate([src, arange_n])
    dst_a = np.concatenate([dst, arange_n])

    # ---- per-core segment counts ----
    core_of = dst_a // NPC
    tloc = (dst_a - core_of * NPC) // P          # 0..97
    qq = src_a // QR                              # 0..3
    segkey = tloc * NQ + qq                       # 0..391

    counts = np.zeros((NCORES, TPC, NQ), np.int64)
    per_core = []
    for c in range(NCORES):
        m = core_of == c
        sk = segkey[m]
        counts[c] = np.bincount(sk, minlength=TPC * NQ).reshape(TPC, NQ)
        # sort by (segment, src) so each chunk's gather rows are
        # address-ordered: better HBM locality for the row gathers
        src_loc_all = src_a[m] - qq[m] * QR
        order = np.lexsort((src_loc_all, sk))
        s_l = (src_a[m][order] - qq[m][order] * QR).astype(np.int16)
        d_l = (dst_a[m][order] - c * NPC - tloc[m][order] * P).astype(np.float32)
        seg_off = np.zeros(TPC * NQ + 1, np.int64)
        np.cumsum(counts[c].reshape(-1), out=seg_off[1:])
        per_core.append((s_l, d_l, seg_off))

    cch = _ceil_div(counts.max(axis=0), P)        # [TPC, NQ] common chunk counts

    # ---- common structural schedule ----
    # chunk order: st-major, then q, then tile. one seg = (t, q) block of chunks
    seg_chunk_off = np.zeros((TPC, NQ), np.int64)
    chunk_tile = []        # global chunk -> tile
    calls = []             # dicts: st, q, ch0, nch, slot0, col0
    ch = 0
    col = 0
    for st in range(NST):
        t_lo, t_hi = G * st, min(G * st + G, TPC)
        for q in range(NQ):
            cc = int(cch[t_lo:t_hi, q].sum())
            if cc == 0:
                continue
            for t in range(t_lo, t_hi):
                seg_chunk_off[t, q] = ch + int(cch[t_lo:t, q].sum())
            tiles_seq = np.repeat(
                np.arange(t_lo, t_hi), cch[t_lo:t_hi, q]
            )
            chunk_tile.extend(tiles_seq.tolist())
            sub0 = 0
            while sub0 < cc:
                n = min(CAP, cc - sub0)
                calls.append(
                    dict(st=st, q=q, ch0=ch + sub0, nch=n, col0=col)
                )
                col += n * 8  # n*128 slots / 16
                sub0 += n
            ch += cc
    totch = ch
    idxcols = col
    chunk_tile = np.array(chunk_tile, np.int64)

    first_ch = np.full(TPC, -1, np.int64)
    last_ch = np.full(TPC, -1, np.int64)
    for k, t in enumerate(chunk_tile):
        if first_ch[t] < 0:
            first_ch[t] = k
        last_ch[t] = k

    # ---- per-core slot arrays ----
    idx16_list, dstloc_list = [], []
    pack_calls = []
    for cal in calls:
        pack_calls.append((cal["ch0"] * P, cal["nch"] * P, cal["col0"]))
    for c in range(NCORES):
        s_l, d_l, seg_off = per_core[c]
        slot_idx = np.zeros(totch * P, np.int16)
        slot_dst = np.full(totch * P, -1, np.int8)
        for t in range(TPC):
            for q in range(NQ):
                n = int(counts[c, t, q])
                if n == 0:
                    continue
                so = int(seg_chunk_off[t, q]) * P
                o0 = int(seg_off[t * NQ + q])
                slot_idx[so : so + n] = s_l[o0 : o0 + n]
                slot_dst[so : so + n] = d_l[o0 : o0 + n]
        idx16_list.append(_pack_idx16(slot_idx, pack_calls, idxcols))
        dstloc_list.append(np.ascontiguousarray(slot_dst.reshape(totch, P).T))

    # ---- per-core dense inputs ----
    import ml_dtypes
    xT_list, disT_list = [], []
    for c in range(NCORES):
        xc = np.zeros((TPC * P, IN_C), np.float32)
        xc[:NPC] = x[c * NPC : (c + 1) * NPC]
        xT_list.append(
            np.ascontiguousarray(xc.T).astype(ml_dtypes.bfloat16))
        dd = np.ones(TPC * P, np.float32)
        dd[:NPC] = dis[c * NPC : (c + 1) * NPC]
        disT_list.append(np.ascontiguousarray(dd.reshape(TPC, P).T))

    # ---- eval pairs ----
    ei = old2new[
        np.concatenate(
            [np.asarray(pos_edge_index, np.int64),
             np.asarray(neg_edge_index, np.int64)],
            axis=1,
        )
    ]
    ev_per_core = []
    ev_counts = np.zeros((NCORES, NQ * NQ), np.int64)
    for c in range(NCORES):
        a = ei[0, c * NEVAL_PC : (c + 1) * NEVAL_PC]
        b = ei[1, c * NEVAL_PC : (c + 1) * NEVAL_PC]
        g = (a // QR) * NQ + (b // QR)
        order = np.argsort(g, kind="stable")
        ev_counts[c] = np.bincount(g, minlength=NQ * NQ)
        ev_per_core.append((a[order], b[order], order, g[order]))

    ech = _ceil_div(ev_counts.max(axis=0), P)     # [16]
    ev_goff = np.zeros(NQ * NQ + 1, np.int64)
    np.cumsum(ech, out=ev_goff[1:])
    etotch = int(ev_goff[-1])

    # a-calls: contiguous per qa; b-calls: contiguous per (qa, qb)
    acalls, bcalls = [], []
    acol = bcol = 0
    for qa in range(NQ):
        ch0 = int(ev_goff[qa * NQ])
        ch1 = int(ev_goff[(qa + 1) * NQ])
        sub = ch0
        while sub < ch1:
            n = min(CAP_EV, ch1 - sub)
            acalls.append(dict(q=qa, ch0=sub, nch=n, col0=acol))
            acol += n * 8
            sub += n
        for qb in range(NQ):
            g0 = int(ev_goff[qa * NQ + qb])
            g1 = int(ev_goff[qa * NQ + qb + 1])
            sub = g0
            while sub < g1:
                n = min(CAP_EV, g1 - sub)
                bcalls.append(dict(q=qb, ch0=sub, nch=n, col0=bcol))
                bcol += n * 8
                sub += n
    eacols, ebcols = acol, bcol

    ea16_list, eb16_list, evmap_list = [], [], []
    apack = [(c["ch0"] * P, c["nch"] * P, c["col0"]) for c in acalls]
    bpack = [(c["ch0"] * P, c["nch"] * P, c["col0"]) for c in bcalls]
    for c in range(NCORES):
        a_s, b_s, order, g_s = ev_per_core[c]
        slot_a = np.zeros(etotch * P, np.int16)
        slot_b = np.zeros(etotch * P, np.int16)
        evmap = np.full(etotch * P, -1, np.int64)
        n = len(a_s)
        cumstart = np.zeros(NQ * NQ + 1, np.int64)
        np.cumsum(ev_counts[c], out=cumstart[1:])
        pos_in_g = np.arange(n, dtype=np.int64) - cumstart[g_s]
        slots = ev_goff[g_s] * P + pos_in_g
        slot_a[slots] = (a_s - (g_s // NQ) * QR).astype(np.int16)
        slot_b[slots] = (b_s - (g_s % NQ) * QR).astype(np.int16)
        evmap[slots] = c * NEVAL_PC + order
        ea16_list.append(_pack_idx16(slot_a, apack, eacols))
        eb16_list.append(_pack_idx16(slot_b, bpack, ebcols))
        evmap_list.append(evmap)

    meta = dict(
        cch=cch, calls=calls, totch=totch, idxcols=idxcols,
        chunk_tile=chunk_tile, first_ch=first_ch, last_ch=last_ch,
        seg_chunk_off=seg_chunk_off,
        acalls=acalls, bcalls=bcalls, etotch=etotch,
        eacols=eacols, ebcols=ebcols,
    )
    percore = dict(
        idx16=idx16_list, dstloc=dstloc_list, xT=xT_list, disT=disT_list,
        ea16=ea16_list, eb16=eb16_list, evmap=evmap_list,
    )
    return meta, percore, dis


# ======================================================================
# program build
# ======================================================================

def _build_program(meta, stage="full"):
    totch = meta["totch"]
    idxcols = meta["idxcols"]
    etotch = meta["etotch"]
    lay = _blob_layout(meta)
    CB = lay["_total"]

    nc = bacc.Bacc("TRN2", target_bir_lowering=False, debug=False,
                   num_devices=NCORES)

    blob_in = nc.dram_tensor("blob", [P, CB], I16, kind="ExternalInput")

    logits_out = nc.dram_tensor("logits", [P, etotch], F32, kind="ExternalOutput")
    if stage != "full":
        dbg_out = nc.dram_tensor("dbg", [2 * P, HID_C], F32,
                                 kind="ExternalOutput")

    with tile.TileContext(nc) as tc:
        with (
            tc.tile_pool(name="const", bufs=1) as cst,
            tc.tile_pool(name="sb", bufs=2) as sb,
            tc.tile_pool(name="ps", bufs=2, space="PSUM") as ps,
            tc.tile_pool(name="dram", bufs=1, space="DRAM") as dr,
        ):
            nc.gpsimd.load_library(library_config.mlp)

            # ---------------- constants (from packed blob) ----------------
            from concourse.ap import AP as _AP

            _ctn = [0]
            _cst_tile_orig = cst.tile

            def _ctile(shape, dtype, **kw):
                # unique tag per const tile: unique slot, no ring aliasing
                _ctn[0] += 1
                kw.setdefault("tag", f"cst{_ctn[0]}")
                kw.setdefault("name", f"cst{_ctn[0]}")
                return _cst_tile_orig(shape, dtype, **kw)

            cst.tile = _ctile

            ident = cst.tile([P, P], F32)
            make_identity(nc, ident[:])
            iota_t = cst.tile([P, P], F32)
            nc.gpsimd.iota(iota_t[:], [[1, P]], channel_multiplier=0,
                           allow_small_or_imprecise_dtypes=True)

            disT_t = cst.tile([P, TPC], F32)
            if "disTzero" in _os.environ.get("BISECT2", ""):
                nc.vector.memset(disT_t[:], 1.0)
            else:
                nc.sync.dma_start(
                    out=disT_t[:],
                    in_=blob_in[:, lay["disT"] : lay["disT"] + 2 * TPC]
                    .bitcast(F32))

            _bis = _os.environ.get("BISECT2", "")
            dstloc_t = cst.tile([P, totch], F32)
            if "dstzero" in _bis:
                nc.vector.memset(dstloc_t[:], 0.0)
            else:
                dstloc8_t = cst.tile([P, totch], I8)
                nc.sync.dma_start(
                    out=dstloc8_t[:],
                    in_=blob_in[:, lay["dstloc"] : lay["dstloc"] + totch // 2]
                    .bitcast(I8))
                nc.vector.tensor_copy(out=dstloc_t[:], in_=dstloc8_t[:])

            def load_folded(name, cols):
                # blob region [P, cols/8] holds [16, cols] folded by column
                # chunks; replicate into all 8 partition groups of [P, cols].
                t = cst.tile([P, cols], I16)
                if _os.environ.get("BISECT", "") == "idx1dma":
                    nc.sync.dma_start(out=t[:], in_=blob_in[:, 0:cols])
                    return t
                w = cols // 8
                for g in range(8):
                    in_ap = _AP(tensor=blob_in, offset=lay[name],
                                ap=[[CB, 16], [16 * CB, 8], [1, w]])
                    nc.sync.dma_start(out=t[16 * g : 16 * (g + 1), :], in_=in_ap)
                return t

            idx_t = load_folded("idx", idxcols)
            ea_t = load_folded("ea", meta["eacols"])
            eb_t = load_folded("eb", meta["ebcols"])

            def load_w(off, cols):
                if "wf32" in _os.environ.get("BISECT2", ""):
                    t = cst.tile([P, cols], F32)
                    nc.sync.dma_start(
                        out=t[:],
                        in_=blob_in[:, 0 : 2 * cols].bitcast(F32))
                    return t
                t = cst.tile([P, cols], BF16)
                nc.sync.dma_start(
                    out=t[:], in_=blob_in[:, off : off + cols].bitcast(BF16))
                return t

            W1_t = load_w(lay["W1"], HID_C)
            W2a_t = load_w(lay["W2"], HID_C)
            W2b_t = load_w(lay["W2"] + HID_C, HID_C)
            W3a_t = load_w(lay["W3"], OUT_C)
            W3b_t = load_w(lay["W3"] + OUT_C, OUT_C)

            B1_t = cst.tile([P, HID_C], F32)
            B2_t = cst.tile([P, HID_C], F32)
            B3_t = cst.tile([P, OUT_C], F32)
            if "biasmemset" in _bis:
                nc.vector.memset(B1_t[:], 0.0)
                nc.vector.memset(B2_t[:], 0.0)
                nc.vector.memset(B3_t[:], 0.0)
            else:
                bias_bf = cst.tile([P, 2 * HID_C + OUT_C], BF16)
                nc.sync.dma_start(
                    out=bias_bf[:],
                    in_=_AP(tensor=blob_in, offset=lay["bias"],
                            ap=[[0, P], [1, 2 * HID_C + OUT_C]]).bitcast(BF16))
                nc.vector.tensor_copy(out=B1_t[:], in_=bias_bf[:, 0:HID_C])
                nc.vector.tensor_copy(out=B2_t[:],
                                      in_=bias_bf[:, HID_C : 2 * HID_C])
                nc.vector.tensor_copy(
                    out=B3_t[:], in_=bias_bf[:, 2 * HID_C : 2 * HID_C + OUT_C])

            # ---------------- DRAM buffers ----------------
            hh1_sh = dr.tile([NPC, HID_C], GDT)
            hh1_full = dr.tile([N, HID_C], GDT, addr_space="Shared")
            hh2_sh = dr.tile([NPC, HID_C], GDT)
            hh2_full = dr.tile([N, HID_C], GDT, addr_space="Shared")
            hh3_sh = dr.tile([NPC, OUT_C], GDT)
            hh3_full = dr.tile([N, OUT_C], GDT, addr_space="Shared")
            z_sh = dr.tile([NPC, OUT_C], ZDT)
            z_full = dr.tile([N, OUT_C], ZDT, addr_space="Shared")

            def rows_of(t):
                return LAST_ROWS if t == TPC - 1 else P

            # ---------------- phase A: Hhat1 = (X @ W1) * dis ----------------
            _wdt = F32 if "wf32" in _os.environ.get("BISECT2", "") else BF16
            for t in range(TPC):
                lhs = sb.tile([P, P], _wdt, tag="lhsA", bufs=4)
                if _wdt == F32:
                    nc.sync.dma_start(
                        out=lhs[:], in_=blob_in[:, 0 : 2 * P].bitcast(F32))
                else:
                    nc.sync.dma_start(
                        out=lhs[:],
                        in_=blob_in[:, lay["xT"] + t * P :
                                    lay["xT"] + (t + 1) * P].bitcast(BF16))
                hp = ps.tile([P, HID_C], F32, space="PSUM", tag="hp")
                nc.tensor.matmul(out=hp[:], lhsT=lhs[:],
                                 rhs=W1_t[:], start=True, stop=True)
                hh = sb.tile([P, HID_C], GDT, tag="hh", bufs=4)
                nc.scalar.activation(out=hh[:], in_=hp[:],
                                     func=mybir.ActivationFunctionType.Copy,
                                     scale=disT_t[:, t : t + 1])
                r = rows_of(t)
                nc.sync.dma_start(out=hh1_sh[t * P : t * P + r, :], in_=hh[:r, :])

            if stage != "a0":
                nc.gpsimd.collective_compute(
                    "AllGather", mybir.AluOpType.bypass,
                    ins=[hh1_sh.opt()], outs=[hh1_full.opt()],
                    replica_groups=[list(range(NCORES))],
                )


            def dump_dbg(full_tile, fw):
                pass

            if stage == "a":
                dump_dbg(hh1_full, HID_C)

            # ---------------- aggregation phases ----------------
            def agg_phase(table_full, f_l, b_t, relu, w_next, b_next_f,
                          hh_next_sh, z_mode, st_limit=NST):
                """One aggregation sweep over all supertiles.

                table_full: gather table [N, f_l]; b_t: bias bcast tile;
                relu: apply relu after bias; w_next: (Wa, Wb) tiles or None;
                hh_next_sh: output shard DRAM (next Hhat or z).
                """
                calls = meta["calls"]
                chunk_tile = meta["chunk_tile"]
                first_ch = meta["first_ch"]
                last_ch = meta["last_ch"]
                ci = 0
                for st in range(st_limit):
                    t_lo, t_hi = G * st, min(G * st + G, TPC)
                    aggp = {}
                    for t in range(t_lo, t_hi):
                        aggp[t] = ps.tile([P, f_l], F32, space="PSUM",
                                          tag="agg", bufs=G, name=f"agg{st}_{t}")
                    _ab = _os.environ.get("ABLATE", "")
                    while ci < len(calls) and calls[ci]["st"] == st:
                        cal = calls[ci]
                        nch = cal["nch"]
                        msg = sb.tile([P, nch, f_l], GDT, tag="msg", bufs=4,
                                      name=f"msg{ci}")
                        q0 = cal["q"] * QR
                        if _ab == "compute":
                            # bulk-load ablation: same bytes, contiguous, no SWDGE
                            for j in range(nch):
                                nc.sync.dma_start(
                                    out=msg[:, j, :],
                                    in_=table_full[q0 + j * P : q0 + (j + 1) * P, :])
                        else:
                            nc.gpsimd.dma_gather(
                                out_ap=msg[:],
                                in_ap=table_full[q0 : q0 + QR, :],
                                idxs_ap=idx_t[:, cal["col0"] : cal["col0"] + nch * 8],
                                num_idxs=nch * P,
                                num_idxs_reg=nch * P,
                                elem_size=f_l,
                            )
                        for j in range(nch):
                            if _ab == "gather":
                                break
                            k = cal["ch0"] + j
                            t = int(chunk_tile[k])
                            oh = sb.tile([P, P], GDT, tag="oh", bufs=16,
                                         name=f"oh{k}")
                            nc.vector.tensor_scalar(
                                out=oh[:], in0=iota_t[:],
                                scalar1=dstloc_t[:, k : k + 1], scalar2=None,
                                op0=mybir.AluOpType.is_equal,
                            )
                            nc.tensor.matmul(
                                out=aggp[t][:], lhsT=oh[:], rhs=msg[:, j, :],
                                start=(k == int(first_ch[t])),
                                stop=(k == int(last_ch[t])),
                            )
                        ci += 1
                    for t in range(t_lo, t_hi):
                        if _ab == "gather":
                            break
                        r = rows_of(t)
                        dis_col = disT_t[:, t : t + 1]
                        xp = sb.tile([P, f_l], F32, tag="xp", bufs=2,
                                     name=f"xp{t}")
                        nc.vector.scalar_tensor_tensor(
                            out=xp[:], in0=aggp[t][:], scalar=dis_col,
                            in1=b_t[:], op0=mybir.AluOpType.mult,
                            op1=mybir.AluOpType.add,
                        )
                        if z_mode:
                            if ZDT != F32:
                                zt = sb.tile([P, f_l], ZDT, tag="zt", bufs=2,
                                             name=f"zt{t}")
                                nc.scalar.activation(
                                    out=zt[:], in_=xp[:],
                                    func=mybir.ActivationFunctionType.Copy,
                                )
                                nc.sync.dma_start(
                                    out=hh_next_sh[t * P : t * P + r, :],
                                    in_=zt[:r, :],
                                )
                            else:
                                nc.sync.dma_start(
                                    out=hh_next_sh[t * P : t * P + r, :],
                                    in_=xp[:r, :],
                                )
                            continue
                        xr = sb.tile([P, f_l], F32, tag="xr", bufs=2,
                                     name=f"xr{t}")
                        if relu:
                            nc.scalar.activation(
                                out=xr[:], in_=xp[:],
                                func=mybir.ActivationFunctionType.Relu,
                            )
                        else:
                            nc.vector.tensor_copy(out=xr[:], in_=xp[:])
                        # transpose xr -> xT blocks, then H_next = xr @ W_next
                        nblk = f_l // P
                        xT2 = sb.tile([P, nblk * P],
                                      F32 if "wf32" in _os.environ.get(
                                          "BISECT2", "") else BF16,
                                      tag="xT2", bufs=2, name=f"xT2{t}")
                        for b2 in range(nblk):
                            tp = ps.tile([P, P], F32, space="PSUM", tag="tp",
                                         bufs=2, name=f"tp{t}_{b2}")
                            nc.tensor.transpose(
                                out=tp[:], in_=xr[:, b2 * P : (b2 + 1) * P],
                                identity=ident[:],
                            )
                            nc.scalar.activation(
                                out=xT2[:, b2 * P : (b2 + 1) * P], in_=tp[:],
                                func=mybir.ActivationFunctionType.Copy,
                            )
                        hp = ps.tile([P, b_next_f], F32, space="PSUM",
                                     tag="hp", bufs=2, name=f"hpx{t}")
                        for b2 in range(nblk):
                            nc.tensor.matmul(
                                out=hp[:], lhsT=xT2[:, b2 * P : (b2 + 1) * P],
                                rhs=w_next[b2][:],
                                start=(b2 == 0), stop=(b2 == nblk - 1),
                            )
                        hh = sb.tile([P, b_next_f], GDT, tag="hh", bufs=4,
                                     name=f"hhx{t}")
                        nc.scalar.activation(
                            out=hh[:], in_=hp[:],
                            func=mybir.ActivationFunctionType.Copy,
                            scale=dis_col,
                        )
                        nc.sync.dma_start(
                            out=hh_next_sh[t * P : t * P + r, :], in_=hh[:r, :]
                        )

            if stage == "b1":
                agg_phase(hh1_full, HID_C, B1_t, True, (W2a_t, W2b_t), HID_C,
                          hh2_sh, False, st_limit=1)
                nc.sync.dma_start(out=dbg_out[0:P, :], in_=hh2_sh[0:P, :])
                nc.sync.dma_start(out=dbg_out[P : 2 * P, :],
                                  in_=hh2_sh[P : 2 * P, :])

            # layer1 agg + H2
            if stage in ("b", "c", "d", "full"):
                agg_phase(hh1_full, HID_C, B1_t, True, (W2a_t, W2b_t), HID_C,
                          hh2_sh, False)
                nc.gpsimd.collective_compute(
                    "AllGather", mybir.AluOpType.bypass,
                    ins=[hh2_sh.opt()], outs=[hh2_full.opt()],
                    replica_groups=[list(range(NCORES))],
                )
                if stage == "b":
                    dump_dbg(hh2_full, HID_C)
            # layer2 agg + H3
            if stage in ("c", "d", "full"):
                agg_phase(hh2_full, HID_C, B2_t, True, (W3a_t, W3b_t), OUT_C,
                          hh3_sh, False)
                nc.gpsimd.collective_compute(
                    "AllGather", mybir.AluOpType.bypass,
                    ins=[hh3_sh.opt()], outs=[hh3_full.opt()],
                    replica_groups=[list(range(NCORES))],
                )
                if stage == "c":
                    dump_dbg(hh3_full, OUT_C)
            # layer3 agg -> z
            if stage in ("d", "full"):
                agg_phase(hh3_full, OUT_C, B3_t, False, None, OUT_C, z_sh, True)
                nc.gpsimd.collective_compute(
                    "AllGather", mybir.AluOpType.bypass,
                    ins=[z_sh.opt()], outs=[z_full.opt()],
                    replica_groups=[list(range(NCORES))],
                )
                if stage == "d":
                    dump_dbg(z_full, OUT_C)

            # ---------------- decode ----------------
            if stage == "full":
                _build_decode(nc, tc, cst, sb, meta, z_full, ea_t, eb_t,
                              logits_out)

    nc.compile()
    return nc


def _build_decode(nc, tc, cst, sb, meta, z_full, ea_t, eb_t, logits_out):
    etotch = meta["etotch"]
    if True:
        if True:
            logits_t = sb.tile([P, etotch], F32, tag="logits", bufs=1)
            acalls = meta["acalls"]
            bcalls = meta["bcalls"]
            ai = bi = -1
            za = zb = None
            zacal = zbcal = None
            prod = None
            for k in range(etotch):
                if ai + 1 < len(acalls) and acalls[ai + 1]["ch0"] == k:
                    ai += 1
                    zacal = acalls[ai]
                    za = sb.tile([P, zacal["nch"], OUT_C], ZDT, tag="za",
                                 bufs=4, name=f"za{ai}")
                    q0 = zacal["q"] * QR
                    nc.gpsimd.dma_gather(
                        out_ap=za[:], in_ap=z_full[q0 : q0 + QR, :],
                        idxs_ap=ea_t[:, zacal["col0"] : zacal["col0"] + zacal["nch"] * 8],
                        num_idxs=zacal["nch"] * P,
                        num_idxs_reg=zacal["nch"] * P,
                        elem_size=OUT_C,
                    )
                if bi + 1 < len(bcalls) and bcalls[bi + 1]["ch0"] == k:
                    bi += 1
                    zbcal = bcalls[bi]
                    zb = sb.tile([P, zbcal["nch"], OUT_C], ZDT, tag="zb",
                                 bufs=4, name=f"zb{bi}")
                    q0 = zbcal["q"] * QR
                    nc.gpsimd.dma_gather(
                        out_ap=zb[:], in_ap=z_full[q0 : q0 + QR, :],
                        idxs_ap=eb_t[:, zbcal["col0"] : zbcal["col0"] + zbcal["nch"] * 8],
                        num_idxs=zbcal["nch"] * P,
                        num_idxs_reg=zbcal["nch"] * P,
                        elem_size=OUT_C,
                    )
                prod = sb.tile([P, OUT_C], F32, tag="prod", bufs=4,
                               name=f"prod{k}")
                nc.vector.tensor_mul(
                    out=prod[:], in0=za[:, k - zacal["ch0"], :],
                    in1=zb[:, k - zbcal["ch0"], :],
                )
                nc.vector.reduce_sum(
                    out=logits_t[:, k : k + 1], in_=prod[:],
                    axis=mybir.AxisListType.X,
                )
            nc.sync.dma_start(out=logits_out[:, :], in_=logits_t[:])


# ======================================================================
# entry point
# ======================================================================

def _pack_blob(meta, percore, c, W1, W2, W3, b1, b2, b3):
    import ml_dtypes

    lay = _blob_layout(meta)
    CB = lay["_total"]
    blob = np.zeros((P, CB), np.int16)

    def put(name, arr_i16):
        blob[:, lay[name] : lay[name] + arr_i16.shape[1]] = arr_i16

    def bf(a):
        return np.ascontiguousarray(
            np.asarray(a, np.float32).astype(ml_dtypes.bfloat16)
        ).view(np.int16)

    put("xT", np.ascontiguousarray(percore["xT"][c]).view(np.int16))
    put("disT", np.ascontiguousarray(percore["disT"][c]).view(np.int16))
    put("dstloc", np.ascontiguousarray(percore["dstloc"][c]).view(np.int16))
    put("idx", _fold8(percore["idx16"][c]))
    put("ea", _fold8(percore["ea16"][c]))
    put("eb", _fold8(percore["eb16"][c]))
    put("W1", bf(W1))
    W2v = bf(W2)
    put("W2", np.concatenate([W2v[:P], W2v[P:]], axis=1))
    W3v = bf(W3)
    put("W3", np.concatenate([W3v[:P], W3v[P:]], axis=1))
    brow = np.concatenate([bf(b1).ravel(), bf(b2).ravel(), bf(b3).ravel()])
    blob[0, lay["bias"] : lay["bias"] + brow.shape[0]] = brow
    return blob


def _make_in_maps(inputs, meta, percore):
    W1 = np.asarray(inputs["W1"], np.float32)
    W2 = np.asarray(inputs["W2"], np.float32)
    W3 = np.asarray(inputs["W3"], np.float32)
    b1 = np.asarray(inputs["b1"], np.float32)
    b2 = np.asarray(inputs["b2"], np.float32)
    b3 = np.asarray(inputs["b3"], np.float32)
    return [
        {"blob": _pack_blob(meta, percore, c, W1, W2, W3, b1, b2, b3)}
        for c in range(NCORES)
    ]


def _run(inputs, trace=False):
    x = np.asarray(inputs["x"], np.float32)
    meta, percore, _dis = _preprocess(
        x, inputs["edge_index"], inputs["pos_edge_index"],
        inputs["neg_edge_index"],
    )
    nc = _build_program(meta)
    in_maps = _make_in_maps(inputs, meta, percore)

    res = run_bass_kernel_spmd(
        nc, in_maps, core_ids=list(range(NCORES)), trace=False
    )

    logits = np.zeros(NCORES * NEVAL_PC, np.float32)
    etotch = meta["etotch"]
    for c in range(NCORES):
        out = res.results[c]["logits"]          # [P, etotch]
        vals = out.T.reshape(-1)                 # slot s = col*128+p -> s = ?
        # slot s -> (p = s % 128, col = s // 128); out.T.ravel() gives
        # [col, p] ordering = slot order
        evmap = percore["evmap"][c]
        valid = evmap >= 0
        logits[evmap[valid]] = vals[valid]
    return logits, res


def kernel(**inputs):
    logits, _ = _run(inputs, trace=False)
    return logits


# ======================================================================
# wall-clock benchmarking (no NTFF hook in this container)
# ======================================================================

def _make_sharded_exec(nc, in_maps, donate=False):
    """Mimic bass2jax.run_bass_via_pjrt's multi-core path but keep the jitted
    callable so repeat executions can be timed with device-resident inputs."""
    import jax
    from jax.sharding import Mesh, PartitionSpec
    from jax.experimental.shard_map import shard_map
    import concourse.mybir as mb
    from concourse.bass2jax import (
        _bass_exec_p, install_neuronx_cc_hook, partition_id_tensor,
    )

    install_neuronx_cc_hook()
    partition_name = (
        nc.partition_id_tensor.name if nc.partition_id_tensor else None
    )
    in_names, out_names, out_avals, zero_outs = [], [], [], []
    for alloc in nc.m.functions[0].allocations:
        if not isinstance(alloc, mb.MemoryLocationSet):
            continue
        name = alloc.memorylocations[0].name
        if alloc.kind == "ExternalInput":
            if name != partition_name:
                in_names.append(name)
        elif alloc.kind == "ExternalOutput":
            out_names.append(name)
            shape = tuple(alloc.tensor_shape)
            dtype = mb.dt.np(alloc.dtype)
            out_avals.append(jax.core.ShapedArray(shape, dtype))
            zero_outs.append(np.zeros(shape, dtype))
    n_params = len(in_names)
    n_outs = len(out_avals)
    in_names.extend(out_names)
    if partition_name is not None:
        in_names.append(partition_name)

    def _body(*args):
        operands = list(args)
        if partition_name is not None:
            operands.append(partition_id_tensor())
        return tuple(_bass_exec_p.bind(
            *operands, out_avals=tuple(out_avals), in_names=tuple(in_names),
            out_names=tuple(out_names), lowering_input_output_aliases=(),
            sim_require_finite=True, sim_require_nnan=True, nc=nc,
        ))

    devices = jax.devices()[:NCORES]
    mesh = Mesh(np.asarray(devices), ("core",))
    in_specs = (PartitionSpec("core"),) * (n_params + n_outs)
    out_specs = (PartitionSpec("core"),) * len(out_names)
    sharded = jax.jit(
        shard_map(_body, mesh=mesh, in_specs=in_specs, out_specs=out_specs,
                  check_rep=False),
        donate_argnums=tuple(range(n_params, n_params + n_outs)) if donate else (),
        keep_unused=True,
    )
    per_core = [[np.asarray(m[name]) for name in in_names[:n_params]]
                for m in in_maps]
    concat_in = [
        np.concatenate([per_core[c][i] for c in range(NCORES)], axis=0)
        for i in range(n_params)
    ]
    concat_zeros = [
        np.zeros((NCORES * z.shape[0], *z.shape[1:]), z.dtype)
        for z in zero_outs
    ]
    dev_in = [jax.device_put(a) for a in concat_in]
    dev_zero = [jax.device_put(z) for z in concat_zeros]
    return sharded, dev_in, dev_zero, out_names, out_avals


def _slope_passes(fn, dev_in, dev_zero, passes=5, nhi=9):
    """Per-exec marginal time: queue nhi execs back-to-back (async dispatch,
    single block) vs 1 exec; slope removes fixed dispatch/RPC latency."""
    import time as _time
    import jax

    outs = fn(*dev_in, *dev_zero)
    jax.block_until_ready(outs)
    slopes = []
    for _ in range(passes):
        t0 = _time.perf_counter()
        outs = fn(*dev_in, *dev_zero)
        jax.block_until_ready(outs)
        t1 = _time.perf_counter() - t0
        t0 = _time.perf_counter()
        for _ in range(nhi):
            outs = fn(*dev_in, *dev_zero)
        jax.block_until_ready(outs)
        thi = _time.perf_counter() - t0
        slopes.append((thi - t1) / (nhi - 1))
    return slopes, outs


def bench(inputs, iters=5):
    """Run + time. Returns (logits, full_slopes, trivial_slopes) in seconds
    per exec (pipelined marginal cost)."""
    x = np.asarray(inputs["x"], np.float32)
    meta, percore, _dis = _preprocess(
        x, inputs["edge_index"], inputs["pos_edge_index"],
        inputs["neg_edge_index"],
    )
    nc = _build_program(meta)
    in_maps = _make_in_maps(inputs, meta, percore)
    fn, dev_in, dev_zero, out_names, out_avals = _make_sharded_exec(nc, in_maps)
    times, outs = _slope_passes(fn, dev_in, dev_zero, passes=iters)

    li = out_names.index("logits")
    etotch = meta["etotch"]
    lo = np.asarray(outs[li]).reshape(NCORES, P, etotch)
    logits = np.zeros(NCORES * NEVAL_PC, np.float32)
    for c in range(NCORES):
        vals = lo[c].T.reshape(-1)
        evmap = percore["evmap"][c]
        valid = evmap >= 0
        logits[evmap[valid]] = vals[valid]

    bl = _baseline_time(iters)
    return logits, times, bl


def _baseline_time(iters=5):
    nc = bacc.Bacc("TRN2", target_bir_lowering=False, debug=False,
                   num_devices=NCORES)
    a_in = nc.dram_tensor("a", [P, P], F32, kind="ExternalInput")
    o_out = nc.dram_tensor("o", [P, P], F32, kind="ExternalOutput")
    with tile.TileContext(nc) as tc:
        with tc.tile_pool(name="sb", bufs=1) as sb:
            t = sb.tile([P, P], F32)
            nc.sync.dma_start(out=t[:], in_=a_in[:, :])
            nc.sync.dma_start(out=o_out[:, :], in_=t[:])
    nc.compile()
    in_maps = [{"a": np.zeros((P, P), np.float32)} for _ in range(NCORES)]
    fn, dev_in, dev_zero, _, _ = _make_sharded_exec(nc, in_maps)
    slopes, _ = _slope_passes(fn, dev_in, dev_zero, passes=iters)
    return slopes

